# revision 5
# baseline (speedup 1.0000x reference)
"""MACG mixture log-likelihood kernel for 8 Trainium2 NeuronCores.

Math (per reference):
  S_k  = lower-triangular 256x256 scatter of S_vec[k]          (host)
  Z    = S_k^T X[n]                (PE matmul, triangular-block skip)
  Y[n] = Z^T Z   (8x8 Gram)        (PE block-diag batched matmul)
  log_pdf[k,n]   = logdet(Y[n])    (batched LDL^T on vector engine)
  density[k,n]   = bias_k - 128*log_pdf
  out = sum_n logsumexp_k density  (vector/scalar engines)

Sharding: data-parallel over N across 8 cores; final scalar reduced on host.
"""

import math
import sys

import numpy as np

for _p in ("/opt/trn_rl_repo",):
    if _p not in sys.path:
        sys.path.insert(0, _p)

import concourse.bass as bass
import concourse.tile as tile
from concourse import bacc, mybir
from concourse import bass_utils
from concourse.bass import ds

K = 16
P = 256
Q = 8
N_TOTAL = 50000
N_CORES = 8
N_PER_CORE = N_TOTAL // N_CORES  # 6250
TILE_N = 128
N_TILES = (N_PER_CORE + TILE_N - 1) // TILE_N  # 49
N_PAD = N_TILES * TILE_N  # 6272
NSUB = 16  # samples per gram stationary (128 partitions / 8 q)
F32 = mybir.dt.float32
F32R = mybir.dt.float32r  # single-pass PE fp32: 4x faster at >=256 moving cols
BF16 = mybir.dt.bfloat16
AF = mybir.ActivationFunctionType
ALU = None  # set after import below
from concourse.alu_op_type import AluOpType as ALU  # noqa: E402


def _log_surface_area(p, q):
    c = p / 2.0
    lg = (q * (q - 1) / 4.0) * math.log(math.pi)
    lg += sum(math.lgamma(c - i / 2.0) for i in range(q))
    return lg - q * math.log(2.0) - (q * p / 2.0) * math.log(math.pi)


def build_program(n_tiles=N_TILES):
    """Builds the single-core Bass program (SPMD across 8 cores)."""
    nc = bacc.Bacc(
        "TRN2",
        target_bir_lowering=False,
        debug=False,
        enable_asserts=False,
        num_devices=N_CORES,
    )
    n_pad = n_tiles * TILE_N

    x_d = nc.dram_tensor("Xp", [n_pad, P, Q], F32R, kind="ExternalInput").ap()
    s_d = nc.dram_tensor("S", [K, P, P], F32R, kind="ExternalInput").ap()
    bias_d = nc.dram_tensor("bias128", [TILE_N, K], F32, kind="ExternalInput").ap()
    e_d = nc.dram_tensor("E", [TILE_N, Q], BF16, kind="ExternalInput").ap()
    mask_d = nc.dram_tensor("mask", [TILE_N, 512], BF16, kind="ExternalInput").ap()
    valid_d = nc.dram_tensor("valid", [n_pad, 1], F32, kind="ExternalInput").ap()
    out_d = nc.dram_tensor("out_acc", [TILE_N, 1], F32, kind="ExternalOutput").ap()

    with tile.TileContext(nc) as tc:
        with (
            tc.tile_pool(name="const", bufs=1) as const,
            tc.tile_pool(name="xt", bufs=2) as xtp,
            tc.tile_pool(name="z", bufs=2) as zp,
            tc.tile_pool(name="g", bufs=3) as gp,
            tc.tile_pool(name="c", bufs=3) as cp,
            tc.tile_pool(name="y", bufs=2) as yp,
            tc.tile_pool(name="sm", bufs=3) as sm,
            tc.tile_pool(name="pzu", bufs=2, space="PSUM") as pzu,
            tc.tile_pool(name="pzl", bufs=2, space="PSUM") as pzl,
            tc.tile_pool(name="pg", bufs=2, space="PSUM") as pgp,
            tc.tile_pool(name="pc", bufs=2, space="PSUM") as pcp,
            tc.tile_pool(name="dram", bufs=2, space="DRAM") as dp,
        ):
            # ---- constants ----
            wts = []  # wts[k] = (W1, W2, W3) stationary blocks of S_k
            for k in range(K):
                w1 = const.tile([128, 128], F32R, tag=f"w1_{k}")
                w2 = const.tile([128, 128], F32R, tag=f"w2_{k}")
                w3 = const.tile([128, 128], F32R, tag=f"w3_{k}")
                nc.sync.dma_start(out=w1, in_=s_d[k, 0:128, 0:128])
                nc.sync.dma_start(out=w2, in_=s_d[k, 128:256, 0:128])
                nc.sync.dma_start(out=w3, in_=s_d[k, 128:256, 128:256])
                wts.append((w1, w2, w3))
            bias_sb = const.tile([TILE_N, K], F32, tag="bias")
            nc.sync.dma_start(out=bias_sb, in_=bias_d)
            e_sb = const.tile([TILE_N, Q], BF16, tag="esel")
            nc.sync.dma_start(out=e_sb, in_=e_d)
            mask_sb = const.tile([TILE_N, 512], BF16, tag="mask")
            nc.sync.dma_start(out=mask_sb, in_=mask_d)
            acc = const.tile([TILE_N, 1], F32, tag="acc")
            nc.vector.memset(acc, 0.0)

            for t in range(n_tiles):
                # ---- load X tile transposed: xt_up/lo [r=128, n=128, q=8] ----
                xt_up = xtp.tile([128, TILE_N, Q], F32R, tag="xtup")
                xt_lo = xtp.tile([128, TILE_N, Q], F32R, tag="xtlo")
                nc.sync.dma_start(
                    out=xt_up,
                    in_=x_d[ds(t * TILE_N, TILE_N), 0:128, :].transpose([1, 0, 2]),
                )
                nc.sync.dma_start(
                    out=xt_lo,
                    in_=x_d[ds(t * TILE_N, TILE_N), 128:256, :].transpose([1, 0, 2]),
                )
                valid_t = sm.tile([TILE_N, 1], F32, tag="valid")
                nc.sync.dma_start(out=valid_t, in_=valid_d[ds(t * TILE_N, TILE_N), :])

                scratch = dp.tile([TILE_N, K, Q, Q], F32, tag="scr")

                for k in range(K):
                    w1, w2, w3 = wts[k]
                    # ---- Z = S_k^T X : [256, (n 128, q 8)] ----
                    # fp32r: single-pass fp32 matmul (full PE rate at 512 cols)
                    z_up = zp.tile([128, TILE_N, Q], BF16, tag="zup")
                    z_lo = zp.tile([128, TILE_N, Q], BF16, tag="zlo")
                    for g in range(2):
                        cs = ds(g * 64, 64)
                        zups = pzu.tile([128, 512], F32, tag="pzup")
                        zlos = pzl.tile([128, 512], F32, tag="pzlo")
                        nc.tensor.matmul(
                            zups,
                            w1,
                            xt_up[:, cs, :],
                            start=True,
                            stop=False,
                        )
                        nc.tensor.matmul(
                            zups,
                            w2,
                            xt_lo[:, cs, :],
                            start=False,
                            stop=True,
                        )
                        nc.tensor.matmul(
                            zlos,
                            w3,
                            xt_lo[:, cs, :],
                            start=True,
                            stop=True,
                        )
                        zups_v = zups[:].rearrange("p (n q) -> p n q", q=Q)
                        zlos_v = zlos[:].rearrange("p (n q) -> p n q", q=Q)
                        nc.scalar.copy(out=z_up[:, cs, :], in_=zups_v)
                        nc.vector.tensor_copy(out=z_lo[:, cs, :], in_=zlos_v)

                    # ---- Gram + compact per wave of 4 subgroups ----
                    callk = cp.tile([Q, 2, 4, NSUB, Q], F32, tag="callk")
                    for w in range(2):
                        gps = pgp.tile([128, 512], F32, tag="pg")
                        for sl in range(4):
                            s = w * 4 + sl
                            mslc = ds(s * NSUB, NSUB)
                            nc.tensor.matmul(
                                gps[:, ds(sl * 128, 128)],
                                z_up[:, mslc, :],
                                z_up[:, mslc, :],
                                start=True,
                                stop=False,
                            )
                            nc.tensor.matmul(
                                gps[:, ds(sl * 128, 128)],
                                z_lo[:, mslc, :],
                                z_lo[:, mslc, :],
                                start=False,
                                stop=True,
                            )
                        gsb = gp.tile([128, 512], BF16, tag="gsb")
                        nc.vector.tensor_copy(out=gsb, in_=gps)
                        gm = gp.tile([128, 512], BF16, tag="gm")
                        nc.gpsimd.tensor_mul(gm, gsb, mask_sb)
                        cps = pcp.tile([Q, 512], F32, tag="pc")
                        nc.tensor.matmul(cps, e_sb, gm, start=True, stop=True)
                        nc.vector.tensor_copy(
                            out=callk[:, w],
                            in_=cps[:].rearrange("p (sl n q) -> p sl n q", sl=4, q=Q),
                        )
                    # scratch[n, k, i, j] ; callk is [i, (w, sl, n'), j]
                    nc.sync.dma_start(
                        out=scratch[:, k, :, :].rearrange(
                            "(w sl np) i j -> i w sl np j", w=2, sl=4
                        ),
                        in_=callk,
                    )

                # ---- LDL^T on Y [128, K, 8, 8] ----
                y4 = yp.tile([TILE_N, K, Q, Q], F32, tag="y4")
                nc.sync.dma_start(out=y4, in_=scratch)
                logd = sm.tile([TILE_N, K, Q], F32, tag="logd")
                for j in range(Q - 1):
                    m = Q - 1 - j
                    r16 = sm.tile([TILE_N, K], F32, tag="rinv")
                    nc.vector.reciprocal(r16, y4[:, :, j, j])
                    nc.scalar.activation(logd[:, :, j], y4[:, :, j, j], func=AF.Ln)
                    wt = sm.tile([TILE_N, K, Q - 1], F32, tag="wt")
                    nc.vector.tensor_tensor(
                        wt[:, :, 0:m],
                        y4[:, :, j + 1 :, j],
                        r16[:, :].unsqueeze(2).to_broadcast((TILE_N, K, m)),
                        op=ALU.mult,
                    )
                    tmp = sm.tile([TILE_N, K, Q - 1, Q - 1], F32, tag="tmp")
                    nc.vector.tensor_tensor(
                        tmp[:, :, 0:m, 0:m],
                        wt[:, :, 0:m].unsqueeze(3).to_broadcast((TILE_N, K, m, m)),
                        y4[:, :, j + 1 :, j].unsqueeze(2).to_broadcast(
                            (TILE_N, K, m, m)
                        ),
                        op=ALU.mult,
                    )
                    nc.vector.tensor_sub(
                        y4[:, :, j + 1 :, j + 1 :],
                        y4[:, :, j + 1 :, j + 1 :],
                        tmp[:, :, 0:m, 0:m],
                    )
                nc.scalar.activation(
                    logd[:, :, Q - 1], y4[:, :, Q - 1, Q - 1], func=AF.Ln
                )
                logpdf = sm.tile([TILE_N, K], F32, tag="logpdf")
                nc.vector.reduce_sum(logpdf[:], logd[:], axis=mybir.AxisListType.X)

                # ---- density + logsumexp over k + accumulate ----
                dens = sm.tile([TILE_N, K], F32, tag="dens")
                nc.vector.scalar_tensor_tensor(
                    dens[:], logpdf[:], -float(P // 2), bias_sb[:],
                    op0=ALU.mult, op1=ALU.add,
                )
                mx = sm.tile([TILE_N, 1], F32, tag="mx")
                nc.vector.reduce_max(mx[:], dens[:], axis=mybir.AxisListType.X)
                tm = sm.tile([TILE_N, K], F32, tag="tm")
                nc.vector.tensor_sub(
                    tm[:], dens[:], mx[:].to_broadcast((TILE_N, K))
                )
                ex = sm.tile([TILE_N, K], F32, tag="ex")
                se = sm.tile([TILE_N, 1], F32, tag="se")
                nc.scalar.activation(ex[:], tm[:], func=AF.Exp, accum_out=se[:])
                ls = sm.tile([TILE_N, 1], F32, tag="ls")
                nc.scalar.activation(ls[:], se[:], func=AF.Ln)
                nc.vector.tensor_add(ls[:], ls[:], mx[:])
                nc.vector.tensor_mul(ls[:], ls[:], valid_t[:])
                nc.vector.tensor_add(acc[:], acc[:], ls[:])

            nc.sync.dma_start(out=out_d, in_=acc)

    nc.compile()
    return nc


def make_host_inputs(X, S_vec, pi, n_tiles=N_TILES):
    """Returns list of 8 per-core input dicts."""
    n_pad = n_tiles * TILE_N
    ti0, ti1 = np.tril_indices(P)
    S = np.zeros((K, P, P), dtype=np.float32)
    S[:, ti0, ti1] = np.asarray(S_vec, dtype=np.float32)

    i_ = np.arange(1, P + 1)
    diag_idx = (i_ * i_ + i_) // 2 - 1
    sv64 = np.asarray(S_vec, dtype=np.float64)
    log_det_s = -2.0 * np.sum(np.log(np.abs(sv64[:, diag_idx])), axis=1)
    pi64 = np.asarray(pi, dtype=np.float64)
    logsm = pi64 - (np.log(np.sum(np.exp(pi64 - pi64.max()))) + pi64.max())
    bias = _log_surface_area(P, Q) - (Q / 2.0) * log_det_s + logsm
    bias128 = np.tile(bias.astype(np.float32)[None, :], (TILE_N, 1))

    import ml_dtypes

    e_mat = np.tile(np.eye(Q, dtype=np.float32), (NSUB, 1)).astype(ml_dtypes.bfloat16)
    mask128 = np.kron(np.eye(NSUB, dtype=np.float32), np.ones((Q, Q), np.float32))
    mask512 = np.tile(mask128, (1, 4)).astype(ml_dtypes.bfloat16)

    X = np.asarray(X, dtype=np.float32)
    in_maps = []
    for c in range(N_CORES):
        xc = X[c * N_PER_CORE : (c + 1) * N_PER_CORE]
        nvalid = min(xc.shape[0], n_pad)
        xp = np.empty((n_pad, P, Q), dtype=np.float32)
        xp[:nvalid] = xc[:nvalid]
        xp[nvalid:] = xc[0]  # safe finite padding; masked out below
        valid = np.zeros((n_pad, 1), dtype=np.float32)
        valid[:nvalid] = 1.0
        in_maps.append(
            {
                "Xp": xp,
                "S": S,
                "bias128": bias128,
                "E": e_mat,
                "mask": mask512,
                "valid": valid,
            }
        )
    return in_maps


_CACHED = {}


def _get_program(n_tiles=N_TILES):
    if n_tiles not in _CACHED:
        _CACHED[n_tiles] = build_program(n_tiles)
    return _CACHED[n_tiles]


def run(X, S_vec, pi, trace=False, n_tiles=N_TILES):
    nc = _get_program(n_tiles)
    in_maps = make_host_inputs(X, S_vec, pi, n_tiles)
    res = bass_utils.run_bass_kernel_spmd(
        nc, in_maps, core_ids=list(range(N_CORES)), trace=trace
    )
    outs = res.results
    total = 0.0
    for c in range(N_CORES):
        total += np.sum(outs[c]["out_acc"].astype(np.float64))
    return np.float32(total), res


def kernel(X, S_vec, pi):
    val, _ = run(X, S_vec, pi)
    return np.array(val, dtype=np.float32)


if __name__ == "__main__":
    # smoke test with random data
    rng = np.random.default_rng(0)
    X = rng.standard_normal((N_TOTAL, P, Q), dtype=np.float32)
    S_vec = rng.standard_normal((K, P * (P + 1) // 2), dtype=np.float32)
    pi = rng.standard_normal((K,), dtype=np.float32)
    print(kernel(X, S_vec, pi))
